# revision 3
# baseline (speedup 1.0000x reference)
"""CRF loss (shared-'I-' IE topology) for Trainium2, data-parallel over batch.

Math notes
----------
reference() loss = (num - den).sum() / num_tokens with, per batch row b:

  num_b = sum_valid_t lp[b,t,y_t] + lsm0[y_0]
          + sum_{t,t-1 both valid} lsmA[y_{t-1}, y_t] + lsmA[y_last, C]

  den_b: the 2-state forward scan
      alpha <- where(m_t, [a + L_t, a + lp0_t], alpha),  a = logaddexp(alpha0, alpha1)
    telescopes exactly (same logaddexp chain, reassociated):
      a_{k+1} = a_k + logaddexp(L_{t_k}, lp0_{t_k}) = a_k + z_{t_k}
      den_b   = sum_{valid t} z_t - z_{t_last} + L_{t_last}
    where z_t = logsumexp_c lp[b,t,:] and L_t = logsumexp_{c>=1} lp[b,t,c].

The memory-bound term is sum_valid z_t (touches all of log_probs, 100 MB).
The device kernel computes s_t = sum_c exp(lp[b,t,:]) for every (b,t) row,
sharded 8 batch rows per core: stream big bf16 chunks (double-buffered
HWDGE DMAs), exp on ACT (the throughput floor: 1 elem/cycle/partition),
then a halving-tree of bf16 tensor_adds on DVE (the packed-2-byte 4x perf
mode; a plain reduce_sum runs 4-5x slower) down to width 6, one small
reduce_sum to f32, and DMA the [128, rows/128] exp-sums back. The ln and
the validity masking happen on host in float64 (O(B*T), same class as the
numerator label-gathers below). Everything else is O(B*T) label gathers
and O(C^2) tables, done on host; the per-core partials are combined on
host (the all-reduce of the scalar loss from the sharding hint).
"""

import numpy as np
from contextlib import ExitStack

B, T, C = 64, 8192, 48
NCORES = 8
BP = B // NCORES          # batch rows per core
RPP = BP * T // 128       # (b,t) rows per partition (512)
# per-chunk rows-per-partition; small edge chunks shorten pipeline fill/tail
KTS = [32, 64, 64, 64, 64, 64, 64, 64, 32]
assert sum(KTS) == RPP
IGNORE = -100

_cache = {}


def _build_bass():
    import concourse.bacc as bacc
    import concourse.tile as tile
    from concourse import mybir

    nc = bacc.Bacc(name="crf_den")
    # lp ships as bf16: the denominator tolerates it (loss rel-err impact
    # ~3e-6, measured) and it halves the streamed bytes -> 2x DMA speedup.
    lp = nc.dram_tensor("lp", [BP * T, C], mybir.dt.bfloat16, kind="ExternalInput")
    sden = nc.dram_tensor("sden", [128, RPP], mybir.dt.float32, kind="ExternalOutput")

    X = mybir.AxisListType.X
    F32 = mybir.dt.float32
    BF16 = mybir.dt.bfloat16
    with tile.TileContext(nc) as tc, ExitStack() as ctx:
        xp = ctx.enter_context(tc.tile_pool(name="x", bufs=3))
        ep = ctx.enter_context(tc.tile_pool(name="e", bufs=3))
        h1p = ctx.enter_context(tc.tile_pool(name="h1", bufs=2))
        h2p = ctx.enter_context(tc.tile_pool(name="h2", bufs=2))
        sp = ctx.enter_context(tc.tile_pool(name="s", bufs=1))

        s_all = sp.tile([128, RPP], F32)
        with nc.allow_low_precision(reason="bf16 exp-sum tree; ln on host in f64"):
            r0 = 0
            j0 = 0
            for kt in KTS:
                x = xp.tile([128, kt * C], BF16)
                src = lp[r0 : r0 + 128 * kt, :].rearrange("(p k) c -> p (k c)", p=128)
                nc.sync.dma_start(out=x, in_=src)
                e = ep.tile([128, kt * C], BF16)
                nc.scalar.activation(out=e, in_=x, func=mybir.ActivationFunctionType.Exp)
                ev = e.rearrange("p (k c) -> p k c", c=C)
                h1 = h1p.tile([128, kt * 24], BF16)
                h1v = h1.rearrange("p (k c) -> p k c", c=24)
                nc.vector.tensor_add(out=h1v, in0=ev[:, :, 0:24], in1=ev[:, :, 24:48])
                h2 = h2p.tile([128, kt * 12], BF16)
                h2v = h2.rearrange("p (k c) -> p k c", c=12)
                nc.vector.tensor_add(out=h2v, in0=h1v[:, :, 0:12], in1=h1v[:, :, 12:24])
                nc.vector.reduce_sum(s_all[:, j0 : j0 + kt], h2v, axis=X)
                r0 += 128 * kt
                j0 += kt
        nc.sync.dma_start(out=sden[:, :], in_=s_all)
    nc.compile()
    return nc


def _get_nc():
    if "nc" not in _cache:
        _cache["nc"] = _build_bass()
    return _cache["nc"]


def _log_softmax(x, axis=-1):
    m = x.max(axis=axis, keepdims=True)
    return x - m - np.log(np.exp(x - m).sum(axis=axis, keepdims=True))


def _cell_rows():
    """row index (within a core shard) held by cell (p, j) of sden."""
    if "cell_rows" not in _cache:
        rows = np.empty((128, RPP), np.int64)
        r0 = j0 = 0
        p = np.arange(128)[:, None]
        for kt in KTS:
            k = np.arange(kt)[None, :]
            rows[:, j0 : j0 + kt] = r0 + p * kt + k
            r0 += 128 * kt
            j0 += kt
        _cache["cell_rows"] = rows
    return _cache["cell_rows"]


def _make_cached_runner(nc):
    """Cached jitted shard_map over the 8 cores — the same NEFF pipeline that
    run_bass_kernel_spmd's axon path uses (bass2jax._bass_exec_p), but reusable
    across kernel() calls so we don't re-trace/re-jit every invocation."""
    import jax
    from jax.sharding import Mesh, NamedSharding, PartitionSpec
    from jax.experimental.shard_map import shard_map
    from concourse import bass2jax, mybir

    bass2jax.install_neuronx_cc_hook()
    partition_name = nc.partition_id_tensor.name if nc.partition_id_tensor else None

    in_names, out_names, out_avals, zero_outs = [], [], [], []
    for alloc in nc.m.functions[0].allocations:
        if not isinstance(alloc, mybir.MemoryLocationSet):
            continue
        name = alloc.memorylocations[0].name
        if alloc.kind == "ExternalInput":
            if name != partition_name:
                in_names.append(name)
        elif alloc.kind == "ExternalOutput":
            out_names.append(name)
            shape = tuple(alloc.tensor_shape)
            dtype = mybir.dt.np(alloc.dtype)
            out_avals.append(jax.core.ShapedArray(shape, dtype))
            zero_outs.append(np.zeros(shape, dtype))
    n_params = len(in_names)
    all_names = list(in_names) + list(out_names)
    if partition_name is not None:
        all_names.append(partition_name)

    def _body(*args):
        operands = list(args)
        if partition_name is not None:
            operands.append(bass2jax.partition_id_tensor())
        return tuple(
            bass2jax._bass_exec_p.bind(
                *operands,
                out_avals=tuple(out_avals),
                in_names=tuple(all_names),
                out_names=tuple(out_names),
                lowering_input_output_aliases=(),
                sim_require_finite=True,
                sim_require_nnan=True,
                nc=nc,
            )
        )

    devices = jax.devices()[:NCORES]
    mesh = Mesh(np.asarray(devices), ("core",))
    in_specs = (PartitionSpec("core"),) * (n_params + len(out_names))
    out_specs = (PartitionSpec("core"),) * len(out_names)
    fn = jax.jit(
        shard_map(_body, mesh=mesh, in_specs=in_specs, out_specs=out_specs,
                  check_rep=False),
        keep_unused=True,
    )
    sharding = NamedSharding(mesh, PartitionSpec("core"))
    zeros_full = [
        np.zeros((NCORES * z.shape[0], *z.shape[1:]), z.dtype) for z in zero_outs
    ]

    def run(in_concat: dict):
        import jax as _jax

        args = [_jax.device_put(in_concat[n], sharding) for n in in_names]
        args += [_jax.device_put(z, sharding) for z in zeros_full]
        outs = fn(*args)
        return {
            name: np.asarray(outs[i]).reshape(NCORES, *out_avals[i].shape)
            for i, name in enumerate(out_names)
        }

    return run


def _warmup_devices():
    """A tiny op per device re-establishes terminal state after a transient
    NRT_EXEC_UNIT_UNRECOVERABLE wedge."""
    import jax

    for d in jax.devices()[:NCORES]:
        try:
            jax.block_until_ready(
                jax.numpy.sum(jax.device_put(np.ones(8, np.float32), d))
            )
        except Exception:
            pass


def _run_device(lp):
    """Per-row s = sum_c exp(lp).  Returns (B*T,) f64."""
    import time as _time
    import ml_dtypes

    lp2 = np.ascontiguousarray(
        lp.reshape(B * T, C).astype(ml_dtypes.bfloat16)  # RNE; den-only
    )

    def _via_runner():
        if "runner" not in _cache:
            _cache["runner"] = _make_cached_runner(_get_nc())
        return _cache["runner"]({"lp": lp2})["sden"]

    def _via_spmd():
        from concourse.bass_utils import run_bass_kernel_spmd

        in_maps = [
            {"lp": lp2[ci * BP * T : (ci + 1) * BP * T]} for ci in range(NCORES)
        ]
        res = run_bass_kernel_spmd(_get_nc(), in_maps, core_ids=list(range(NCORES)))
        return np.stack([r["sden"] for r in res.results])

    s_per_core = None  # [NCORES, 128, RPP]
    attempts = [_via_runner, _via_runner, _via_spmd, _via_runner, _via_spmd]
    backoff = [5.0, 15.0, 30.0, 45.0]
    for i, attempt in enumerate(attempts):
        try:
            s_per_core = attempt()
            break
        except Exception:
            if i == len(attempts) - 1:
                raise
            _cache.pop("runner", None)
            _time.sleep(backoff[min(i, len(backoff) - 1)])
            _warmup_devices()

    rows = _cell_rows()  # [128, RPP] row index within a core shard
    s_rows = np.empty(B * T, np.float64)
    for ci in range(NCORES):
        s_rows[ci * BP * T + rows] = s_per_core[ci].astype(np.float64)
    return s_rows


def kernel(**inputs):
    lp = np.ascontiguousarray(np.asarray(inputs["log_probs"], dtype=np.float32))
    labels_in = np.asarray(inputs["labels"])
    A_start = np.asarray(inputs["A_start"], dtype=np.float64)
    A_trans = np.asarray(inputs["A_trans"], dtype=np.float64)
    labels = labels_in.astype(np.int32).reshape(B, T)

    s_rows = _run_device(lp)

    mask = labels != IGNORE
    lengths = mask.sum(axis=1)
    y = np.where(mask, labels, 0).astype(np.intp)

    lsm0 = _log_softmax(A_start)
    lsmA = _log_softmax(A_trans, axis=-1)

    emis = np.take_along_axis(lp, y[..., None], axis=2)[..., 0].astype(np.float64)
    num_emis = (emis * mask).sum(axis=1)
    tmask = mask[:, 1:] & mask[:, :-1]
    num_trans = lsm0[y[:, 0]] + (lsmA[y[:, :-1], y[:, 1:]] * tmask).sum(axis=1)
    last_idx = np.clip(lengths - 1, 0, T - 1)
    y_last = y[np.arange(B), last_idx]
    num = num_emis + num_trans + lsmA[y_last, C]

    # masked z-sum from the device exp-sums; ln in f64 on host
    s_bt = np.maximum(s_rows.reshape(B, T), 1e-300)  # guard log(0)*0 -> nan
    zsum = np.where(mask, np.log(s_bt), 0.0).sum(axis=1)

    rows_last = lp[np.arange(B), last_idx, :].astype(np.float64)  # (B, 48)
    mx = rows_last.max(axis=1, keepdims=True)
    z_last = (mx + np.log(np.exp(rows_last - mx).sum(axis=1, keepdims=True)))[:, 0]
    r1 = rows_last[:, 1:]
    mx1 = r1.max(axis=1, keepdims=True)
    L_last = (mx1 + np.log(np.exp(r1 - mx1).sum(axis=1, keepdims=True)))[:, 0]
    den = np.where(lengths > 0, zsum - z_last + L_last, 0.0)

    loss = (num - den).sum() / lengths.sum()
    return np.float32(loss)
